# revision 9
# baseline (speedup 1.0000x reference)
"""CrissCrossAttention Trainium2 kernel (8 NeuronCores, data-parallel).

Problem: B=4, C=256, H=W=128, 4 heads. Per head: cq=8 q/k channels, cv=64
v channels. Row attention (over W per row) + column attention (over H per
column), outputs added with the CCNet spatial-transpose quirk, then
out = gamma*attn + x.

Sharding: 16 (batch, head) pairs over 8 cores -> each core handles
batch b = core//2 and head pair p = core%2 (global heads 2p, 2p+1).
Each core reads x[b] (all 256 channels, needed by the projections) and
produces output channels [128p : 128p+128] of batch b.

Core-local pipeline (pixels indexed pix = h*128 + w):
  - qk projection -> flat row-major [32, h*128+w] and col-major
    [32, w*128+h] bf16 stores (bias fused into the PSUM evacuation).
  - band-packed operand stores for the PE (matmul operands must start at
    32-aligned partitions): q/k value for row h lives at partition
    32*(h%4)+c -> the 4 rows of a group occupy distinct PE row-groups and
    their K=8 energy matmuls run concurrently via tile_position.
    Built from the flat stores with SBUF->SBUF DMAs (off-engine).
  - vT projection (pixel-major): vT[128w, 128h, 130]; 130 free cols =
    [64 v-chans head0 | 1 | 64 v-chans head1 | 1]; Wv, bv pre-scaled by
    gamma on host; the ones column gives the softmax denominator
    (unscaled) for free in the PV matmul.
  - vTc = spatial transpose of vT via DMA xbar transposes (bf16).
  - Per row r, head hh:  eT[v,w] = k^T q (PE, K=8);  pT = exp(eT) (ACT,
    no max subtraction -- logits are O(10), safe in fp32);
    o2[w, 0:65] = pT.T @ vT_aug (PE; col 64 = denominator);
    t[w, c] = o2[:, 0:64] * recip(o2[:, 64]).
  - Column attention identical using qc/kc stores and vTc. The CCNet
    transpose aligns row-tile(row i) and col-tile(col i) elementwise on
    output row i, so attn_un[j, c] = t_row(i)[j,c] + t_col(i)[j,c].
  - PE-transpose attn_un to channel-major, add residual x, DMA out.
"""

import os
import numpy as np
from contextlib import ExitStack

import concourse.bass as bass
import concourse.bacc as bacc
import concourse.tile as tile
from concourse import mybir
from concourse.masks import make_identity

F32 = mybir.dt.float32
BF16 = mybir.dt.bfloat16

B, C, H, W = 4, 256, 128, 128
PIX = H * W            # 16384
CV = 64                # v channels per head
NCORES = 8
G = 4                  # rows per attention group (= PE row-group packing)
NG = H // G            # 32 groups


def build_program():
    nc = bacc.Bacc("TRN2", target_bir_lowering=False, debug=False,
                   num_devices=NCORES)

    x_in = nc.dram_tensor("x_in", [C, PIX], F32, kind="ExternalInput")
    x_res = nc.dram_tensor("x_res", [128, PIX], F32, kind="ExternalInput")
    wqkT = nc.dram_tensor("wqkT", [C, 32], F32, kind="ExternalInput")
    qk_bias = nc.dram_tensor("qk_bias", [32, 1], F32, kind="ExternalInput")
    wvT = nc.dram_tensor("wvT", [C, 130], F32, kind="ExternalInput")
    vbias_row = nc.dram_tensor("vbias_row", [1, 130], F32, kind="ExternalInput")
    out = nc.dram_tensor("out", [128, PIX], F32, kind="ExternalOutput")

    with tile.TileContext(nc) as tc, ExitStack() as ctx:
        consts = ctx.enter_context(tc.tile_pool(name="consts", bufs=1))
        persist = ctx.enter_context(tc.tile_pool(name="persist", bufs=1))

        # constants / weights
        wqa = consts.tile([128, 32], F32, tag="wqa")
        wqb = consts.tile([128, 32], F32, tag="wqb")
        nc.sync.dma_start(wqa, wqkT[0:128, :])
        nc.sync.dma_start(wqb, wqkT[128:256, :])
        wva = consts.tile([128, 130], F32, tag="wva")
        wvb = consts.tile([128, 130], F32, tag="wvb")
        nc.sync.dma_start(wva, wvT[0:128, :])
        nc.sync.dma_start(wvb, wvT[128:256, :])
        qkb = consts.tile([32, 1], F32, tag="qkb")
        nc.sync.dma_start(qkb, qk_bias[:, :])
        vbias = consts.tile([1, 130], F32, tag="vbias")
        nc.sync.dma_start(vbias, vbias_row[:, :])
        ones1 = consts.tile([1, 128], F32, tag="ones1")
        nc.vector.memset(ones1, 1.0)
        ident = consts.tile([128, 128], F32, tag="ident")
        make_identity(nc, ident)

        # persistent activations
        # band-packed operand stores: partition 32*(h%4)+c, c<8
        q_sb = persist.tile([128, 2, H // 4, W], BF16, tag="q")    # 16 KiB
        k_sb = persist.tile([128, 2, H // 4, W], BF16, tag="k")    # 16 KiB
        qc_sb = persist.tile([128, 2, W // 4, H], BF16, tag="qc")  # 16 KiB
        kc_sb = persist.tile([128, 2, W // 4, H], BF16, tag="kc")  # 16 KiB
        # c-outer pixel-major value stores: [part, channel, spatial]
        vT_sb = persist.tile([128, 130, H], BF16, tag="vT")        # 32.5 KiB
        vTc_sb = persist.tile([128, 130, W], BF16, tag="vTc")      # 32.5 KiB

        # ---------------- Phase B: projections ----------------
        with (
            tc.tile_pool(name="qkflat", bufs=1) as flatpool,
            tc.tile_pool(name="xchunk", bufs=2) as xpool,
            tc.tile_pool(name="pq", bufs=2, space="PSUM") as pqpool,
            tc.tile_pool(name="pv", bufs=3, space="PSUM") as pvpool,
        ):
            fr = flatpool.tile([32, PIX], BF16, tag="fr")  # [c, h*128+w]
            fc = flatpool.tile([32, PIX], BF16, tag="fc")  # [c, w*128+h]
            # view of fc indexed [c, h, w]
            fc_hw = fc[:, :].rearrange("c (w h) -> c h w", h=H)

            CHUNK = 512  # pixels per chunk = 4 rows
            for chi in range(PIX // CHUNK):
                c0 = chi * CHUNK
                xa = xpool.tile([128, CHUNK], F32, tag="xa")
                xb = xpool.tile([128, CHUNK], F32, tag="xb")
                nc.sync.dma_start(xa, x_in[0:128, c0 : c0 + CHUNK])
                nc.sync.dma_start(xb, x_in[128:256, c0 : c0 + CHUNK])
                # qk projection, 512-pixel (4-row) sub-chunks
                for s in range(CHUNK // 512):
                    pq = pqpool.tile([32, 4, 128], F32, tag="pq")
                    sl = slice(s * 512, (s + 1) * 512)
                    r0 = (c0 + s * 512) // 128
                    nc.tensor.matmul(pq[:, :, :].rearrange("c r w -> c (r w)"),
                                     wqa, xa[:, sl], start=True, stop=False)
                    nc.tensor.matmul(pq[:, :, :].rearrange("c r w -> c (r w)"),
                                     wqb, xb[:, sl], start=False, stop=True)
                    nc.vector.tensor_scalar_add(
                        fr[:, c0 + s * 512 : c0 + (s + 1) * 512], pq, qkb
                    )
                    nc.vector.tensor_scalar_add(
                        fc_hw[:, r0 : r0 + 4, :], pq, qkb
                    )
                # vT projection, one 128-pixel row at a time
                for s in range(CHUNK // 128):
                    r = (c0 + s * 128) // 128
                    pv = pvpool.tile([128, 130], F32, tag="pv")
                    sl = slice(s * 128, (s + 1) * 128)
                    nc.tensor.matmul(pv, xa[:, sl], wva, start=True, stop=False)
                    nc.tensor.matmul(pv, xb[:, sl], wvb, start=False, stop=False)
                    nc.tensor.matmul(pv, ones1, vbias, start=False, stop=True)
                    nc.scalar.copy(vT_sb[:, :, r], pv)

            # band the flat stores (SBUF->SBUF DMA, partition moves)
            # fr [c, (hb b w)] -> q_sb[32b+c, hh, hb, w]
            for bb in range(4 if os.environ.get("K_SKIP_BAND") != "1" else 0):
                for hh in range(2):
                    src_r = fr[:, :].rearrange(
                        "c (hb b w) -> c b hb w", b=4, w=W)
                    src_c = fc[:, :].rearrange(
                        "c (wb b h) -> c b wb h", b=4, h=H)
                    nc.sync.dma_start(
                        q_sb[32 * bb : 32 * bb + 8, hh, :, :],
                        src_r[8 * hh : 8 * hh + 8, bb, :, :])
                    nc.sync.dma_start(
                        k_sb[32 * bb : 32 * bb + 8, hh, :, :],
                        src_r[16 + 8 * hh : 24 + 8 * hh, bb, :, :])
                    nc.sync.dma_start(
                        qc_sb[32 * bb : 32 * bb + 8, hh, :, :],
                        src_c[8 * hh : 8 * hh + 8, bb, :, :])
                    nc.sync.dma_start(
                        kc_sb[32 * bb : 32 * bb + 8, hh, :, :],
                        src_c[16 + 8 * hh : 24 + 8 * hh, bb, :, :])

        # ---------------- Phase B2: vTc via DMA xbar transposes ----------------
        # vT[w, c, h] -> vTc[h, c, w], independently per channel c.
        if os.environ.get("K_SKIP_B2") != "1":
            for cch in range(130):
                nc.sync.dma_start_transpose(vTc_sb[:, cch, :], vT_sb[:, cch, :])

        # ---------------- Phase C: attention ----------------
        with (
            tc.tile_pool(name="pe", bufs=1, space="PSUM") as pepool,
            tc.tile_pool(name="po", bufs=2, space="PSUM") as popool,
            tc.tile_pool(name="pat", bufs=2, space="PSUM") as patpool,
            tc.tile_pool(name="pt", bufs=3) as ptpool,
            tc.tile_pool(name="tt", bufs=3) as tpool,
            tc.tile_pool(name="au", bufs=2) as aupool,
            tc.tile_pool(name="rc", bufs=4) as rcpool,
            tc.tile_pool(name="io", bufs=3) as iopool,
        ):
            STAGE = int(os.environ.get("K_C_STAGE", "6"))
            CNG = int(os.environ.get("K_NG", str(NG)))
            for g in range(CNG if os.environ.get("K_SKIP_C") != "1" else 0):
                t_dir = []
                for d in range(2):  # 0 = row attention, 1 = column attention
                    qs = q_sb if d == 0 else qc_sb
                    ks = k_sb if d == 0 else kc_sb
                    vs = vT_sb if d == 0 else vTc_sb
                    til = tpool.tile([128, G, 2, CV], F32, tag="t")
                    for hh in range(2):
                        # one PSUM bank per concurrent row-group matmul
                        pe = pepool.tile([128, G, 512], F32, tag="pe")
                        for j in range(G):
                            nc.tensor.matmul(
                                pe[:, j, 0:128],
                                ks[32 * j : 32 * j + 8, hh, g, :],
                                qs[32 * j : 32 * j + 8, hh, g, :],
                                start=True, stop=True,
                                tile_position=(32 * j, 0),
                            )
                        if STAGE < 2:
                            continue
                        pT = ptpool.tile([128, G, 128], BF16, tag="pt")
                        nc.scalar.activation(pT, pe[:, :, 0:128],
                                             mybir.ActivationFunctionType.Exp)
                        if STAGE < 3:
                            continue
                        po = popool.tile([128, G, 65], F32, tag="po")
                        for j in range(G):
                            i = g * G + j
                            nc.tensor.matmul(
                                po[:, j, :], pT[:, j, :],
                                vs[:, 65 * hh : 65 * hh + 65, i],
                                start=True, stop=True,
                            )
                        if STAGE < 4:
                            continue
                        rec = rcpool.tile([128, G, 1], F32, tag="rc")
                        nc.vector.reciprocal(rec, po[:, :, 64:65])
                        nc.vector.tensor_tensor(
                            til[:, :, hh, :], po[:, :, 0:64],
                            rec.to_broadcast((128, G, CV)),
                            mybir.AluOpType.mult,
                        )
                    t_dir.append(til)
                if STAGE < 5:
                    continue
                au = aupool.tile([128, G, 128], F32, tag="au")
                nc.vector.tensor_tensor(au, t_dir[0][:, :, :, :],
                                        t_dir[1][:, :, :, :],
                                        mybir.AluOpType.add)
                pat = patpool.tile([128, G, 128], F32, tag="pat")
                for j in range(G):
                    nc.tensor.transpose(pat[:, j, :], au[:, j, :], ident)
                if STAGE < 6:
                    continue
                xres = iopool.tile([128, G * 128], F32, tag="xres")
                nc.sync.dma_start(xres, x_res[:, g * 512 : (g + 1) * 512])
                res = iopool.tile([128, G * 128], F32, tag="res")
                nc.vector.tensor_tensor(
                    res, pat[:, :, :].rearrange("p g w -> p (g w)"),
                    xres, mybir.AluOpType.add)
                nc.sync.dma_start(out[:, g * 512 : (g + 1) * 512], res)

    return nc


def _prep_core_inputs(core, x, Wq, bq, Wk, bk, Wv, bv, gamma):
    b = core // 2
    p = core % 2
    g = float(np.asarray(gamma).reshape(-1)[0])
    qsl = slice(16 * p, 16 * p + 16)
    vsl = slice(128 * p, 128 * p + 128)

    wqk = np.zeros((C, 32), np.float32)
    wqk[:, 0:16] = Wq[qsl].T       # q head even(8) | q head odd(8)
    wqk[:, 16:32] = Wk[qsl].T
    qkb = np.concatenate([bq[qsl], bk[qsl]]).reshape(32, 1).astype(np.float32)

    wv_eff = (g * Wv[vsl]).astype(np.float32)     # [128, 256]
    bv_eff = (g * bv[vsl]).astype(np.float32)
    wvt = np.zeros((C, 130), np.float32)
    wvt[:, 0:64] = wv_eff[0:64].T
    wvt[:, 65:129] = wv_eff[64:128].T
    vbias = np.zeros((1, 130), np.float32)
    vbias[0, 0:64] = bv_eff[0:64]
    vbias[0, 64] = 1.0
    vbias[0, 65:129] = bv_eff[64:128]
    vbias[0, 129] = 1.0

    return {
        "x_in": np.ascontiguousarray(x[b].reshape(C, PIX), np.float32),
        "x_res": np.ascontiguousarray(x[b, vsl].reshape(128, PIX), np.float32),
        "wqkT": wqk,
        "qk_bias": qkb,
        "wvT": wvt,
        "vbias_row": vbias,
    }


_NC_CACHE = None


def _get_nc():
    global _NC_CACHE
    if _NC_CACHE is None:
        nc = build_program()
        nc.compile()
        _NC_CACHE = nc
    return _NC_CACHE


def kernel(x, Wq, bq, Wk, bk, Wv, bv, gamma, _trace=False, _trace_kwargs=None):
    from concourse.bass_utils import run_bass_kernel_spmd

    nc = _get_nc()
    in_maps = [
        _prep_core_inputs(core, x, Wq, bq, Wk, bk, Wv, bv, gamma)
        for core in range(NCORES)
    ]
    res = run_bass_kernel_spmd(
        nc, in_maps, list(range(NCORES)), trace=_trace,
        **(_trace_kwargs or {}),
    )
    outp = np.empty((B, C, H, W), np.float32)
    for core in range(NCORES):
        b, p = core // 2, core % 2
        outp[b, 128 * p : 128 * p + 128] = (
            res.results[core]["out"].reshape(128, H, W)
        )
    if _trace:
        kernel.last_results = res
    return outp


# revision 11
# speedup vs baseline: 1.1119x; 1.1119x over previous
"""CrissCrossAttention Trainium2 kernel (8 NeuronCores, data-parallel).

Problem: B=4, C=256, H=W=128, 4 heads. Per head: cq=8 q/k channels, cv=64
v channels. Row attention (over W per row) + column attention (over H per
column), outputs added with the CCNet spatial-transpose quirk, then
out = gamma*attn + x.

Sharding: 16 (batch, head) pairs over 8 cores -> each core handles
batch b = core//2 and head pair p = core%2 (global heads 2p, 2p+1).
Each core reads x[b] (all 256 channels, needed by the projections) and
produces output channels [128p : 128p+128] of batch b.

Core-local pipeline (pixels indexed pix = h*128 + w):
  - qk projection -> flat row-major fr[32, h*128+w] and col-major
    fc[32, w*128+h] bf16 stores. fc comes from a second matmul pass with a
    column-ordered (strided) moving operand so both evacuations write
    near-contiguously. Bias is fused into the PSUM evacuation.
  - band-packed operand stores for the PE (matmul operands must start at
    32-aligned partitions): q/k value for row h lives at partition
    32*(h%4)+c -> the 4 rows of a group occupy distinct PE row-groups and
    their K=8 energy matmuls run concurrently via tile_position (each into
    its own PSUM bank -- concurrent row-group matmuls must not share one).
    Built from the flat stores with SBUF->SBUF DMAs (off-engine).
  - vT projection (pixel-major): vT[128w, 128h, 128c] bf16, channels =
    [64 head0 | 64 head1], Wv and bv pre-scaled by gamma on host.
  - vTc[h, w, c] = spatial transpose of vT via per-channel PE transposes
    (the DMA xbar path is a single ~26 GB/s unit -- 160 us serial stall).
  - Per row r, head hh:  eT[v,w] = k^T q (PE, K=8, 4 rows concurrent);
    pT = exp(eT) (ACT, no max subtraction -- logits are O(10));
    o2[w, 0:64] = pT.T @ vT slice; o2[w,64] = colsum via ones column
    matmul reusing the same stationary pT (softmax denominator);
    t[w, c] = o2[:, 0:64] * recip(o2[:, 64]) (DVE).
  - Column attention identical using qc/kc stores and vTc. The CCNet
    transpose aligns row-tile(row i) and col-tile(col i) elementwise on
    output row i: attn_un[j, c] = t_row(i)[j,c] + t_col(i)[j,c] (GpSimd).
  - PE-transpose attn_un (bf16) to channel-major, add residual x, DMA out.
"""

import os
import numpy as np
from contextlib import ExitStack

import concourse.bass as bass
import concourse.bacc as bacc
import concourse.tile as tile
from concourse import mybir
from concourse.masks import make_identity

F32 = mybir.dt.float32
BF16 = mybir.dt.bfloat16

B, C, H, W = 4, 256, 128, 128
PIX = H * W            # 16384
CV = 64                # v channels per head
NCORES = 8
G = 4                  # rows per attention group (= PE row-group packing)
NG = H // G            # 32 groups


def build_program():
    nc = bacc.Bacc("TRN2", target_bir_lowering=False, debug=False,
                   num_devices=NCORES)

    x_in = nc.dram_tensor("x_in", [C, PIX], F32, kind="ExternalInput")
    x_res = nc.dram_tensor("x_res", [128, PIX], F32, kind="ExternalInput")
    wqkT = nc.dram_tensor("wqkT", [C, 32], F32, kind="ExternalInput")
    qk_bias = nc.dram_tensor("qk_bias", [32, 1], F32, kind="ExternalInput")
    wvT = nc.dram_tensor("wvT", [C, 128], F32, kind="ExternalInput")
    vbias_row = nc.dram_tensor("vbias_row", [1, 128], F32, kind="ExternalInput")
    out = nc.dram_tensor("out", [128, PIX], F32, kind="ExternalOutput")

    with tile.TileContext(nc) as tc, ExitStack() as ctx:
        consts = ctx.enter_context(tc.tile_pool(name="consts", bufs=1))
        persist = ctx.enter_context(tc.tile_pool(name="persist", bufs=1))

        # constants / weights
        wqa = consts.tile([128, 32], F32, tag="wqa")
        wqb = consts.tile([128, 32], F32, tag="wqb")
        nc.sync.dma_start(wqa, wqkT[0:128, :])
        nc.sync.dma_start(wqb, wqkT[128:256, :])
        wva = consts.tile([128, 128], F32, tag="wva")
        wvb = consts.tile([128, 128], F32, tag="wvb")
        nc.sync.dma_start(wva, wvT[0:128, :])
        nc.sync.dma_start(wvb, wvT[128:256, :])
        qkb = consts.tile([32, 1], F32, tag="qkb")
        nc.sync.dma_start(qkb, qk_bias[:, :])
        vbias = consts.tile([1, 128], F32, tag="vbias")
        nc.sync.dma_start(vbias, vbias_row[:, :])
        ones1 = consts.tile([1, 128], F32, tag="ones1")
        nc.vector.memset(ones1, 1.0)
        onecol = consts.tile([128, 1], BF16, tag="onecol")
        nc.vector.memset(onecol, 1.0)
        identb = consts.tile([128, 128], BF16, tag="identb")
        make_identity(nc, identb)

        # persistent activations
        # band-packed operand stores: partition 32*(h%4)+c, c<8
        q_sb = persist.tile([128, 2, H // 4, W], BF16, tag="q")    # 16 KiB
        k_sb = persist.tile([128, 2, H // 4, W], BF16, tag="k")    # 16 KiB
        qc_sb = persist.tile([128, 2, W // 4, H], BF16, tag="qc")  # 16 KiB
        kc_sb = persist.tile([128, 2, W // 4, H], BF16, tag="kc")  # 16 KiB
        # pixel-major value stores, channel innermost
        vT_sb = persist.tile([128, H, 128], BF16, tag="vT")        # 32 KiB
        vTc_sb = persist.tile([128, W, 128], BF16, tag="vTc")      # 32 KiB

        # ---------------- Phase B: projections ----------------
        with (
            tc.tile_pool(name="qkflat", bufs=1) as flatpool,
            tc.tile_pool(name="xchunk", bufs=3) as xpool,
            tc.tile_pool(name="pq", bufs=2, space="PSUM") as pqpool,
            tc.tile_pool(name="pqc", bufs=2, space="PSUM") as pqcpool,
            tc.tile_pool(name="pv", bufs=3, space="PSUM") as pvpool,
        ):
            fr = flatpool.tile([32, PIX], BF16, tag="fr")  # [c, h*128+w]
            fc = flatpool.tile([32, PIX], BF16, tag="fc")  # [c, w*128+h]

            CHUNK = 512  # pixels per chunk = 4 rows
            NCH = PIX // CHUNK
            for chi in range(NCH):
                c0 = chi * CHUNK
                r0 = c0 // 128
                eng = nc.sync if chi % 2 == 0 else nc.scalar
                xa = xpool.tile([128, CHUNK], F32, tag="xa")
                xb = xpool.tile([128, CHUNK], F32, tag="xb")
                eng.dma_start(xa, x_in[0:128, c0 : c0 + CHUNK])
                eng.dma_start(xb, x_in[128:256, c0 : c0 + CHUNK])
                xav = xa[:, :].rearrange("p (r w) -> p r w", w=128)
                xbv = xb[:, :].rearrange("p (r w) -> p r w", w=128)

                # qk projection, row-pixel order
                pq = pqpool.tile([32, 512], F32, tag="pq")
                nc.tensor.matmul(pq, wqa, xa[:, :], start=True, stop=False)
                nc.tensor.matmul(pq, wqb, xb[:, :], start=False, stop=True)
                nc.vector.tensor_scalar_add(fr[:, c0 : c0 + CHUNK], pq, qkb)

                # qk projection again in column-pixel order (strided rhs)
                # -> fc evacuation gets near-contiguous writes
                pqc = pqcpool.tile([32, 128, 4], F32, tag="pqc")
                nc.tensor.matmul(
                    pqc[:, :, :], wqa, xav.rearrange("p r w -> p w r"),
                    start=True, stop=False)
                nc.tensor.matmul(
                    pqc[:, :, :], wqb, xbv.rearrange("p r w -> p w r"),
                    start=False, stop=True)
                fcv = fc[:, :].rearrange("c (w h) -> c w h", h=H)
                nc.vector.tensor_scalar_add(
                    fcv[:, :, r0 : r0 + 4], pqc, qkb)

                # vT projection: 4 rows into one PSUM bank
                pv = pvpool.tile([128, 4, 128], F32, tag="pv")
                for s in range(4):
                    nc.tensor.matmul(pv[:, s, :], xav[:, s, :], wva,
                                     start=True, stop=False)
                    nc.tensor.matmul(pv[:, s, :], xbv[:, s, :], wvb,
                                     start=False, stop=False)
                    nc.tensor.matmul(pv[:, s, :], ones1, vbias,
                                     start=False, stop=True)
                nc.scalar.copy(vT_sb[:, r0 : r0 + 4, :], pv)

            # band the flat stores (SBUF->SBUF DMA, partition moves)
            # fr [c, (hb b w)] -> q_sb[32b+c, hh, hb, w]
            for bb in range(4):
                for hh in range(2):
                    src_r = fr[:, :].rearrange(
                        "c (hb b w) -> c b hb w", b=4, w=W)
                    src_c = fc[:, :].rearrange(
                        "c (wb b h) -> c b wb h", b=4, h=H)
                    eng = nc.sync if hh == 0 else nc.scalar
                    eng.dma_start(
                        q_sb[32 * bb : 32 * bb + 8, hh, :, :],
                        src_r[8 * hh : 8 * hh + 8, bb, :, :])
                    eng.dma_start(
                        k_sb[32 * bb : 32 * bb + 8, hh, :, :],
                        src_r[16 + 8 * hh : 24 + 8 * hh, bb, :, :])
                    eng.dma_start(
                        qc_sb[32 * bb : 32 * bb + 8, hh, :, :],
                        src_c[8 * hh : 8 * hh + 8, bb, :, :])
                    eng.dma_start(
                        kc_sb[32 * bb : 32 * bb + 8, hh, :, :],
                        src_c[16 + 8 * hh : 24 + 8 * hh, bb, :, :])

        # ---------------- Phase B2: vTc via PE transposes ----------------
        # vT[w, h, c] -> vTc[h, w, c]; per channel, batched 4 per bank.
        with tc.tile_pool(name="ptr", bufs=2, space="PSUM") as ptrpool:
            for cb in range(32):
                ptr = ptrpool.tile([128, 4, 128], BF16, tag="ptr")
                for cj in range(4):
                    cch = cb * 4 + cj
                    nc.tensor.matmul(ptr[:, cj, :], vT_sb[:, :, cch], identb,
                                     start=True, stop=True, is_transpose=True)
                nc.vector.tensor_copy(
                    vTc_sb[:, :, cb * 4 : cb * 4 + 4],
                    ptr[:, :, :].rearrange("p c w -> p w c"))

        # ---------------- Phase C: attention ----------------
        with (
            tc.tile_pool(name="pe", bufs=1, space="PSUM") as pepool,
            tc.tile_pool(name="po", bufs=2, space="PSUM") as popool,
            tc.tile_pool(name="pat", bufs=2, space="PSUM") as patpool,
            tc.tile_pool(name="pt", bufs=3) as ptpool,
            tc.tile_pool(name="tt", bufs=3) as tpool,
            tc.tile_pool(name="au", bufs=2) as aupool,
            tc.tile_pool(name="rc", bufs=4) as rcpool,
            tc.tile_pool(name="io", bufs=3) as iopool,
        ):
            for g in range(NG):
                t_dir = []
                for d in range(2):  # 0 = row attention, 1 = column attention
                    qs = q_sb if d == 0 else qc_sb
                    ks = k_sb if d == 0 else kc_sb
                    vs = vT_sb if d == 0 else vTc_sb
                    til = tpool.tile([128, G, 2, CV], BF16, tag="t")
                    for hh in range(2):
                        # one PSUM bank per concurrent row-group matmul
                        pe = pepool.tile([128, G, 512], F32, tag="pe")
                        for j in range(G):
                            nc.tensor.matmul(
                                pe[:, j, 0:128],
                                ks[32 * j : 32 * j + 8, hh, g, :],
                                qs[32 * j : 32 * j + 8, hh, g, :],
                                start=True, stop=True,
                                tile_position=(32 * j, 0),
                            )
                        pT = ptpool.tile([128, G, 128], BF16, tag="pt")
                        nc.scalar.activation(pT, pe[:, :, 0:128],
                                             mybir.ActivationFunctionType.Exp)
                        po = popool.tile([128, G, 65], F32, tag="po")
                        for j in range(G):
                            i = g * G + j
                            nc.tensor.matmul(
                                po[:, j, 0:64], pT[:, j, :],
                                vs[:, i, 64 * hh : 64 * hh + 64],
                                start=True, stop=True,
                            )
                            nc.tensor.matmul(
                                po[:, j, 64:65], pT[:, j, :], onecol,
                                start=True, stop=True,
                            )
                        rec = rcpool.tile([128, G, 1], F32, tag="rc")
                        nc.vector.reciprocal(rec, po[:, :, 64:65])
                        nc.vector.tensor_tensor(
                            til[:, :, hh, :], po[:, :, 0:64],
                            rec.to_broadcast((128, G, CV)),
                            mybir.AluOpType.mult,
                        )
                    t_dir.append(til)
                au = aupool.tile([128, G, 128], BF16, tag="au")
                nc.gpsimd.tensor_tensor(au, t_dir[0][:, :, :, :],
                                        t_dir[1][:, :, :, :],
                                        mybir.AluOpType.add)
                pat = patpool.tile([128, G, 128], BF16, tag="pat")
                for j in range(G):
                    nc.tensor.matmul(pat[:, j, :], au[:, j, :], identb,
                                     start=True, stop=True, is_transpose=True)
                eng = nc.sync if g % 2 == 0 else nc.scalar
                xres = iopool.tile([128, G * 128], F32, tag="xres")
                eng.dma_start(xres, x_res[:, g * 512 : (g + 1) * 512])
                res = iopool.tile([128, G * 128], F32, tag="res")
                nc.vector.tensor_tensor(
                    res, pat[:, :, :].rearrange("p g w -> p (g w)"),
                    xres, mybir.AluOpType.add)
                eng.dma_start(out[:, g * 512 : (g + 1) * 512], res)

    return nc


def _prep_core_inputs(core, x, Wq, bq, Wk, bk, Wv, bv, gamma):
    b = core // 2
    p = core % 2
    g = float(np.asarray(gamma).reshape(-1)[0])
    qsl = slice(16 * p, 16 * p + 16)
    vsl = slice(128 * p, 128 * p + 128)

    wqk = np.zeros((C, 32), np.float32)
    wqk[:, 0:16] = Wq[qsl].T       # q head even(8) | q head odd(8)
    wqk[:, 16:32] = Wk[qsl].T
    qkb = np.concatenate([bq[qsl], bk[qsl]]).reshape(32, 1).astype(np.float32)

    wv_eff = (g * Wv[vsl]).astype(np.float32)     # [128, 256]
    bv_eff = (g * bv[vsl]).astype(np.float32)
    wvt = np.ascontiguousarray(wv_eff.T)          # [256, 128]
    vbias = bv_eff.reshape(1, 128).copy()

    return {
        "x_in": np.ascontiguousarray(x[b].reshape(C, PIX), np.float32),
        "x_res": np.ascontiguousarray(x[b, vsl].reshape(128, PIX), np.float32),
        "wqkT": wqk,
        "qk_bias": qkb,
        "wvT": wvt,
        "vbias_row": vbias,
    }


_NC_CACHE = None


def _get_nc():
    global _NC_CACHE
    if _NC_CACHE is None:
        nc = build_program()
        nc.compile()
        _NC_CACHE = nc
    return _NC_CACHE


def kernel(x, Wq, bq, Wk, bk, Wv, bv, gamma, _trace=False, _trace_kwargs=None):
    from concourse.bass_utils import run_bass_kernel_spmd

    nc = _get_nc()
    in_maps = [
        _prep_core_inputs(core, x, Wq, bq, Wk, bk, Wv, bv, gamma)
        for core in range(NCORES)
    ]
    res = run_bass_kernel_spmd(
        nc, in_maps, list(range(NCORES)), trace=_trace,
        **(_trace_kwargs or {}),
    )
    outp = np.empty((B, C, H, W), np.float32)
    for core in range(NCORES):
        b, p = core // 2, core % 2
        outp[b, 128 * p : 128 * p + 128] = (
            res.results[core]["out"].reshape(128, H, W)
        )
    if _trace:
        kernel.last_results = res
    return outp


# revision 13
# speedup vs baseline: 1.4984x; 1.3475x over previous
"""CrissCrossAttention Trainium2 kernel (8 NeuronCores, data-parallel).

Problem: B=4, C=256, H=W=128, 4 heads. Per head: cq=8 q/k channels, cv=64
v channels. Row attention (over W per row) + column attention (over H per
column), outputs added with the CCNet spatial-transpose quirk, then
out = gamma*attn + x.

Sharding: 16 (batch, head) pairs over 8 cores -> each core handles
batch b = core//2 and head pair p = core%2 (global heads 2p, 2p+1).
Each core reads x[b] (all 256 channels, needed by the projections) and
produces output channels [128p : 128p+128] of batch b.

Core-local pipeline (pixels indexed pix = h*128 + w):
  - qk projection -> flat row-major fr[32, h*128+w] and col-major
    fc[32, w*128+h] bf16 stores. fc comes from a second matmul pass with a
    column-ordered (strided) moving operand so both evacuations write
    near-contiguously. Bias is fused into the PSUM evacuation.
  - band-packed operand stores for the PE (matmul operands must start at
    32-aligned partitions): q/k value for row h lives at partition
    32*(h%4)+c -> the 4 rows of a group occupy distinct PE row-groups and
    their K=8 energy matmuls run concurrently via tile_position (each into
    its own PSUM bank -- concurrent row-group matmuls must not share one).
    Built from the flat stores with SBUF->SBUF DMAs (off-engine).
  - vT projection (pixel-major): vT[128w, 128h, 128c] bf16, channels =
    [64 head0 | 64 head1], Wv and bv pre-scaled by gamma on host.
  - vTc[h, w, c] = spatial transpose of vT via per-channel PE transposes
    (the DMA xbar path is a single ~26 GB/s unit -- 160 us serial stall).
  - Per row r, head hh:  eT[v,w] = k^T q (PE, K=8, 4 rows concurrent);
    pT = exp(eT) (ACT, no max subtraction -- logits are O(10));
    o2[w, 0:64] = pT.T @ vT slice; o2[w,64] = colsum via ones column
    matmul reusing the same stationary pT (softmax denominator);
    t[w, c] = o2[:, 0:64] * recip(o2[:, 64]) (DVE).
  - Column attention identical using qc/kc stores and vTc. The CCNet
    transpose aligns row-tile(row i) and col-tile(col i) elementwise on
    output row i: attn_un[j, c] = t_row(i)[j,c] + t_col(i)[j,c] (GpSimd).
  - PE-transpose attn_un (bf16) to channel-major, add residual x, DMA out.
"""

import os
import numpy as np
from contextlib import ExitStack

import concourse.bass as bass
import concourse.bacc as bacc
import concourse.tile as tile
from concourse import mybir
from concourse.masks import make_identity

F32 = mybir.dt.float32
BF16 = mybir.dt.bfloat16

B, C, H, W = 4, 256, 128, 128
PIX = H * W            # 16384
CV = 64                # v channels per head
NCORES = 8
G = 4                  # rows per attention group (= PE row-group packing)
NG = H // G            # 32 groups


def build_program():
    nc = bacc.Bacc("TRN2", target_bir_lowering=False, debug=False,
                   num_devices=NCORES)

    x_in = nc.dram_tensor("x_in", [C, PIX], F32, kind="ExternalInput")
    x_res = nc.dram_tensor("x_res", [128, PIX], F32, kind="ExternalInput")
    wqkT = nc.dram_tensor("wqkT", [C, 32], BF16, kind="ExternalInput")
    qk_bias = nc.dram_tensor("qk_bias", [32, 1], F32, kind="ExternalInput")
    wvT = nc.dram_tensor("wvT", [C, 130], BF16, kind="ExternalInput")
    vbias_row = nc.dram_tensor("vbias_row", [1, 130], BF16, kind="ExternalInput")
    out = nc.dram_tensor("out", [128, PIX], F32, kind="ExternalOutput")

    with tile.TileContext(nc) as tc, ExitStack() as ctx:
        consts = ctx.enter_context(tc.tile_pool(name="consts", bufs=1))
        persist = ctx.enter_context(tc.tile_pool(name="persist", bufs=1))

        # constants / weights
        wqa = consts.tile([128, 32], BF16, tag="wqa")
        wqb = consts.tile([128, 32], BF16, tag="wqb")
        nc.sync.dma_start(wqa, wqkT[0:128, :])
        nc.sync.dma_start(wqb, wqkT[128:256, :])
        wva = consts.tile([128, 130], BF16, tag="wva")
        wvb = consts.tile([128, 130], BF16, tag="wvb")
        nc.sync.dma_start(wva, wvT[0:128, :])
        nc.sync.dma_start(wvb, wvT[128:256, :])
        qkb = consts.tile([32, 1], F32, tag="qkb")
        nc.sync.dma_start(qkb, qk_bias[:, :])
        vbias = consts.tile([1, 130], BF16, tag="vbias")
        nc.sync.dma_start(vbias, vbias_row[:, :])
        ones1 = consts.tile([1, 128], BF16, tag="ones1")
        nc.vector.memset(ones1, 1.0)
        identb = consts.tile([128, 128], BF16, tag="identb")
        make_identity(nc, identb)

        # persistent activations
        # band-packed operand stores: partition 32*(h%4)+c, c<8
        q_sb = persist.tile([128, 2, H // 4, W], BF16, tag="q")    # 16 KiB
        k_sb = persist.tile([128, 2, H // 4, W], BF16, tag="k")    # 16 KiB
        qc_sb = persist.tile([128, 2, W // 4, H], BF16, tag="qc")  # 16 KiB
        kc_sb = persist.tile([128, 2, W // 4, H], BF16, tag="kc")  # 16 KiB
        # pixel-major value stores, channel innermost
        vT_sb = persist.tile([128, H, 130], BF16, tag="vT")        # 32.5 KiB
        vTc_sb = persist.tile([128, W, 130], BF16, tag="vTc")      # 32.5 KiB

        # ---------------- Phase B: projections ----------------
        with (
            tc.tile_pool(name="qkflat", bufs=1) as flatpool,
            tc.tile_pool(name="xchunk", bufs=2) as xpool,
            tc.tile_pool(name="pq", bufs=2, space="PSUM") as pqpool,
            tc.tile_pool(name="pv", bufs=4, space="PSUM") as pvpool,
        ):
            fr = flatpool.tile([32, PIX], BF16, tag="fr")  # [c, h*128+w]
            fc = flatpool.tile([32, PIX], BF16, tag="fc")  # [c, w*128+h]

            CHUNK = 512  # pixels per chunk = 4 rows
            NCH = PIX // CHUNK
            for chi in range(NCH):
                c0 = chi * CHUNK
                r0 = c0 // 128
                eng = nc.sync if chi % 2 == 0 else nc.scalar
                xa = xpool.tile([128, CHUNK], F32, tag="xa")
                xb = xpool.tile([128, CHUNK], F32, tag="xb")
                eng.dma_start(xa, x_in[0:128, c0 : c0 + CHUNK])
                eng.dma_start(xb, x_in[128:256, c0 : c0 + CHUNK])
                # bf16 copies: cheaper LDWEIGHTS (FWL) for the matmuls
                xab = xpool.tile([128, CHUNK], BF16, tag="xab")
                xbb = xpool.tile([128, CHUNK], BF16, tag="xbb")
                nc.vector.tensor_copy(xab, xa[:, :])
                nc.vector.tensor_copy(xbb, xb[:, :])
                xav = xab[:, :].rearrange("p (r w) -> p r w", w=128)
                xbv = xbb[:, :].rearrange("p (r w) -> p r w", w=128)

                # qk projection, row-pixel order
                pq = pqpool.tile([32, 512], F32, tag="pq")
                nc.tensor.matmul(pq, wqa, xab[:, :], start=True, stop=False)
                nc.tensor.matmul(pq, wqb, xbb[:, :], start=False, stop=True)
                nc.vector.tensor_scalar_add(fr[:, c0 : c0 + CHUNK], pq, qkb)

                # vT projection: 2 rows per PSUM half-bank tile
                for s2 in range(2):
                    pv = pvpool.tile([128, 2, 130], F32, tag="pv")
                    for s3 in range(2):
                        srow = 2 * s2 + s3
                        nc.tensor.matmul(pv[:, s3, :], xav[:, srow, :], wva,
                                         start=True, stop=False)
                        nc.tensor.matmul(pv[:, s3, :], xbv[:, srow, :], wvb,
                                         start=False, stop=False)
                        nc.tensor.matmul(pv[:, s3, :], ones1, vbias,
                                         start=False, stop=True)
                    nc.scalar.copy(
                        vT_sb[:, r0 + 2 * s2 : r0 + 2 * s2 + 2, :], pv)

            # col-major flat store from row-major one (free-dim permute,
            # off the critical engines)
            frv = fr[:, :].rearrange("c (h w) -> c w h", w=W)
            fcv = fc[:, :].rearrange("c (w h) -> c w h", h=H)
            nc.gpsimd.tensor_copy(fcv, frv)

            # band the flat stores (SBUF->SBUF DMA, partition moves)
            # fr [c, (hb b w)] -> q_sb[32b+c, hh, hb, w]
            for bb in range(4):
                for hh in range(2):
                    src_r = fr[:, :].rearrange(
                        "c (hb b w) -> c b hb w", b=4, w=W)
                    src_c = fc[:, :].rearrange(
                        "c (wb b h) -> c b wb h", b=4, h=H)
                    eng = nc.sync if hh == 0 else nc.scalar
                    eng.dma_start(
                        q_sb[32 * bb : 32 * bb + 8, hh, :, :],
                        src_r[8 * hh : 8 * hh + 8, bb, :, :])
                    eng.dma_start(
                        k_sb[32 * bb : 32 * bb + 8, hh, :, :],
                        src_r[16 + 8 * hh : 24 + 8 * hh, bb, :, :])
                    eng.dma_start(
                        qc_sb[32 * bb : 32 * bb + 8, hh, :, :],
                        src_c[8 * hh : 8 * hh + 8, bb, :, :])
                    eng.dma_start(
                        kc_sb[32 * bb : 32 * bb + 8, hh, :, :],
                        src_c[16 + 8 * hh : 24 + 8 * hh, bb, :, :])

        # ---------------- Phase B2: vTc via PE transposes ----------------
        # vT[w, h, c] -> vTc[h, w, c]; per channel, batched 4 per bank.
        with tc.tile_pool(name="ptr", bufs=2, space="PSUM") as ptrpool:
            for cb in range(33):
                nch = min(4, 130 - cb * 4)
                ptr = ptrpool.tile([128, 4, 128], BF16, tag="ptr")
                for cj in range(nch):
                    cch = cb * 4 + cj
                    nc.tensor.matmul(ptr[:, cj, :], vT_sb[:, :, cch], identb,
                                     start=True, stop=True, is_transpose=True)
                nc.vector.tensor_copy(
                    vTc_sb[:, :, cb * 4 : cb * 4 + nch],
                    ptr[:, 0:nch, :].rearrange("p c w -> p w c"))

        # ---------------- Phase C: attention ----------------
        with (
            tc.tile_pool(name="pe", bufs=1, space="PSUM") as pepool,
            tc.tile_pool(name="po", bufs=2, space="PSUM") as popool,
            tc.tile_pool(name="pat", bufs=2, space="PSUM") as patpool,
            tc.tile_pool(name="pt", bufs=3) as ptpool,
            tc.tile_pool(name="tt", bufs=3) as tpool,
            tc.tile_pool(name="au", bufs=2) as aupool,
            tc.tile_pool(name="rc", bufs=4) as rcpool,
            tc.tile_pool(name="io", bufs=3) as iopool,
        ):
            for g in range(NG):
                t_dir = []
                for d in range(2):  # 0 = row attention, 1 = column attention
                    qs = q_sb if d == 0 else qc_sb
                    ks = k_sb if d == 0 else kc_sb
                    vs = vT_sb if d == 0 else vTc_sb
                    til = tpool.tile([128, G, 2, CV], BF16, tag="t")
                    for hh in range(2):
                        # one PSUM bank per concurrent row-group matmul
                        pe = pepool.tile([128, G, 512], F32, tag="pe")
                        for j in range(G):
                            nc.tensor.matmul(
                                pe[:, j, 0:128],
                                ks[32 * j : 32 * j + 8, hh, g, :],
                                qs[32 * j : 32 * j + 8, hh, g, :],
                                start=True, stop=True,
                                tile_position=(32 * j, 0),
                            )
                        pT = ptpool.tile([128, G, 128], BF16, tag="pt")
                        nc.scalar.activation(pT, pe[:, :, 0:128],
                                             mybir.ActivationFunctionType.Exp)
                        po = popool.tile([128, G, 65], F32, tag="po")
                        for j in range(G):
                            i = g * G + j
                            nc.tensor.matmul(
                                po[:, j, :], pT[:, j, :],
                                vs[:, i, 65 * hh : 65 * hh + 65],
                                start=True, stop=True,
                            )
                        rec = rcpool.tile([128, G, 1], F32, tag="rc")
                        nc.vector.reciprocal(rec, po[:, :, 64:65])
                        nc.vector.tensor_tensor(
                            til[:, :, hh, :], po[:, :, 0:64],
                            rec.to_broadcast((128, G, CV)),
                            mybir.AluOpType.mult,
                        )
                    t_dir.append(til)
                au = aupool.tile([128, G, 128], BF16, tag="au")
                nc.gpsimd.tensor_tensor(au, t_dir[0][:, :, :, :],
                                        t_dir[1][:, :, :, :],
                                        mybir.AluOpType.add)
                pat = patpool.tile([128, G, 128], BF16, tag="pat")
                for j in range(G):
                    nc.tensor.matmul(pat[:, j, :], au[:, j, :], identb,
                                     start=True, stop=True, is_transpose=True)
                eng = nc.sync if g % 2 == 0 else nc.scalar
                xres = iopool.tile([128, G * 128], F32, tag="xres")
                eng.dma_start(xres, x_res[:, g * 512 : (g + 1) * 512])
                res = iopool.tile([128, G * 128], F32, tag="res")
                nc.vector.tensor_tensor(
                    res, pat[:, :, :].rearrange("p g w -> p (g w)"),
                    xres, mybir.AluOpType.add)
                eng.dma_start(out[:, g * 512 : (g + 1) * 512], res)

    return nc


def _prep_core_inputs(core, x, Wq, bq, Wk, bk, Wv, bv, gamma):
    b = core // 2
    p = core % 2
    g = float(np.asarray(gamma).reshape(-1)[0])
    qsl = slice(16 * p, 16 * p + 16)
    vsl = slice(128 * p, 128 * p + 128)

    import ml_dtypes
    bf = ml_dtypes.bfloat16

    wqk = np.zeros((C, 32), np.float32)
    wqk[:, 0:16] = Wq[qsl].T       # q head even(8) | q head odd(8)
    wqk[:, 16:32] = Wk[qsl].T
    wqk = wqk.astype(bf)
    qkb = np.concatenate([bq[qsl], bk[qsl]]).reshape(32, 1).astype(np.float32)

    wv_eff = (g * Wv[vsl]).astype(np.float32)     # [128, 256]
    bv_eff = (g * bv[vsl]).astype(np.float32)
    wvt = np.zeros((C, 130), np.float32)
    wvt[:, 0:64] = wv_eff[0:64].T
    wvt[:, 65:129] = wv_eff[64:128].T
    wvt = wvt.astype(bf)
    vbias = np.zeros((1, 130), np.float32)
    vbias[0, 0:64] = bv_eff[0:64]
    vbias[0, 64] = 1.0
    vbias[0, 65:129] = bv_eff[64:128]
    vbias[0, 129] = 1.0
    vbias = vbias.astype(bf)

    return {
        "x_in": np.ascontiguousarray(x[b].reshape(C, PIX), np.float32),
        "x_res": np.ascontiguousarray(x[b, vsl].reshape(128, PIX), np.float32),
        "wqkT": wqk,
        "qk_bias": qkb,
        "wvT": wvt,
        "vbias_row": vbias,
    }


_NC_CACHE = None


def _get_nc():
    global _NC_CACHE
    if _NC_CACHE is None:
        nc = build_program()
        nc.compile()
        _NC_CACHE = nc
    return _NC_CACHE


def kernel(x, Wq, bq, Wk, bk, Wv, bv, gamma, _trace=False, _trace_kwargs=None):
    from concourse.bass_utils import run_bass_kernel_spmd

    nc = _get_nc()
    in_maps = [
        _prep_core_inputs(core, x, Wq, bq, Wk, bk, Wv, bv, gamma)
        for core in range(NCORES)
    ]
    res = run_bass_kernel_spmd(
        nc, in_maps, list(range(NCORES)), trace=_trace,
        **(_trace_kwargs or {}),
    )
    outp = np.empty((B, C, H, W), np.float32)
    for core in range(NCORES):
        b, p = core // 2, core % 2
        outp[b, 128 * p : 128 * p + 128] = (
            res.results[core]["out"].reshape(128, H, W)
        )
    if _trace:
        kernel.last_results = res
    return outp
